# revision 7
# baseline (speedup 1.0000x reference)
"""Group-quantized linear (fake int4 per-group dequant) GEMV on 8 Trainium2 cores.

Reference computation (all fp32):
    qw = round_half_even(clip(W, -8, 7))            # W in [-8, 7) so clip is identity
    out = (qw.reshape(O, 64, 128) * scales[:, :, None]).reshape(O, O) @ x

Sharding: column-parallel — each core owns a 1024-row slice of W/scales,
x replicated, outputs concatenated (per the tensor-parallel hint).  All
host-side work is pure layout: the per-core weight slice ships fully
permuted to the SBUF tile order [c(128), ch(8), gp(8), o(1024)] so weight
DMA descriptors are large contiguous reads, x ships pre-transposed
[128, 64], scales ship pre-arranged (and duplicated over the hi/lo axis),
and the [128, 8] result is un-permuted on the host — no on-device
transposes at all.

DMA-engine load balancing: descriptors round-robin over the 16 DMA engines
E64..E79 and the pointer RESETS to E64 at every dma_start (measured).  On
this part engine E79 runs ~17% slower than the rest (it doubles as the
dynamic-queue manager), so a uniform split leaves a 13 us straggler tail.
Each chunk therefore ships as:
  A:  gp 0..5, all 128 partitions       -> 128 descs x 24 KiB, uniform
  B0..B7: gp 6..7, partitions 15k..15k+14 -> 15 descs x 8 KiB each: E79 gets
      NONE of these (desc index mod 16 never reaches 15)
  C:  gp 6..7, partitions 120..127, max_dma_last_dim=512 -> 32 descs x 2 KiB,
      uniform
Per-chunk bytes: E64..E78 ~260 KiB, E79 ~196 KiB — matching their measured
25.9 / 21.6 GB/s rates so all engines finish together (~10 us/chunk).

Per-core pipeline (device):
  DVE   : quantize via the fp32 magic-number trick (w + 1.5*2^23) - 1.5*2^23
          == round-half-even exactly for |w| < 2^22, cast to bf16 (exact for
          ints in [-8, 7]); one tensor_scalar per A/B part
  PE    : per (group g, out-chunk oc) matmul acc[:, oc, gp, :2] =
          qwT[128c, 128o].T @ x2[128c, 2] where x2 = [x_hi | x_lo] bf16
          Dekker split of x (fp32-accurate), fp32 PSUM; acc is one ping-pong
          PSUM bank per chunk
  DVE   : per chunk, THREE ops covering all out-chunks at once:
          y[128, oc, gp, 2] = acc * sc2[:, ch]      (PSUM read, one TT)
          partial[128, oc]  = reduce_sum(y, XY)     (hi/lo + group reduction)
          out_acc           = out_acc + partial     (ping-pong buffers)

HBM traffic/core = 32 MiB weights; balanced engines => ~80 us stream floor.
"""

import numpy as np

IN_DIM = 8192
OUT_DIM = 8192
NUM_GROUPS = 64
GROUP_SIZE = 128  # IN_DIM // NUM_GROUPS
N_CORES = 8
PER_OUT = OUT_DIM // N_CORES  # 1024
P = 128

MAGIC = np.float32(12582912.0)  # 1.5 * 2**23: (w + MAGIC) - MAGIC == rint(w)

_cache = {}


def _split_multi_waits(nc):
    """walrus in this container accepts only ONE sync-wait per instruction;
    Tile's tail drain carries one per producer proc. Hoist extras onto
    same-engine NoOps placed immediately before — identical semantics for an
    in-order sequencer."""
    import concourse.mybir as mybir

    uid = 0
    for f in nc.m.functions:
        for blk in f.blocks:
            insts = blk.instructions
            if not any(
                i.sync_info is not None
                and i.sync_info.on_wait
                and len(i.sync_info.on_wait) > 1
                for i in insts
            ):
                continue
            new_insts = []
            for inst in insts:
                si = inst.sync_info
                if si is not None and si.on_wait and len(si.on_wait) > 1:
                    waits = list(si.on_wait)
                    for w in waits[:-1]:
                        uid += 1
                        new_insts.append(
                            mybir.InstNoOp(
                                name=f"I-waitsplit-{uid}",
                                engine=inst.engine,
                                ins=[],
                                outs=[],
                                sync_info=mybir.SyncInfo(on_wait=[w], on_update=[]),
                            )
                        )
                    inst.sync_info = mybir.SyncInfo(
                        on_wait=[waits[-1]], on_update=si.on_update
                    )
                new_insts.append(inst)
            blk.instructions = new_insts
    return nc


def build_nc(
    in_dim=IN_DIM,
    per_out=PER_OUT,
    num_groups=NUM_GROUPS,
    groups_per_chunk=8,
    gp_a=6,  # groups in the uniform A part; gpc - gp_a in the E79-skipping B part
    wa_bufs=4,
    wb_bufs=4,
    split_waits=True,
):
    import concourse.bass as bass
    import concourse.mybir as mybir
    import concourse.tile as tile

    f32 = mybir.dt.float32
    bf16 = mybir.dt.bfloat16
    add = mybir.AluOpType.add
    mult = mybir.AluOpType.mult

    ng = num_groups
    gpc = groups_per_chunk
    n_chunks = ng // gpc
    oc_n = per_out // P  # out-chunks of 128
    gp_b = gpc - gp_a
    assert ng % gpc == 0 and per_out % P == 0 and in_dim == ng * GROUP_SIZE

    nc = bass.Bass()
    # host-permuted weights: wt[c, ch, gp, o] = W[o, ch*1024 + gp*128 + c]
    wt = nc.dram_tensor("wt", [P, n_chunks, gpc, per_out], f32, kind="ExternalInput")
    # host-transposed x: x[c, g] = x_full[g*128 + c]
    x_d = nc.dram_tensor("x", [P, ng], f32, kind="ExternalInput")
    # host-arranged scales, duplicated over the hi/lo axis:
    # sc[p, ch, oc, gp, j] = scales[oc*128 + p, ch*gpc + gp]
    sc_d = nc.dram_tensor("scales", [P, n_chunks, oc_n, gpc, 2], f32, kind="ExternalInput")
    # out[p, oc] = result[oc*128 + p]; host un-permutes
    out_d = nc.dram_tensor("out", [P, oc_n], f32, kind="ExternalOutput")

    with tile.TileContext(nc) as tc:
        with (
            tc.tile_pool(name="singles", bufs=1) as singles,
            tc.tile_pool(name="wa", bufs=wa_bufs) as wapool,
            tc.tile_pool(name="wb", bufs=wb_bufs) as wbpool,
            tc.tile_pool(name="qa", bufs=2) as qapool,
            tc.tile_pool(name="qb", bufs=2) as qbpool,
            tc.tile_pool(name="ep", bufs=2) as epool,
            tc.tile_pool(name="psum", bufs=2, space="PSUM") as psum,
        ):
            # x / scales on the gpsimd DMA queue: its ring is empty at t=0 so
            # these tiny transfers land immediately, in parallel with the
            # sync ring's first weight DMA.
            xT = singles.tile([P, ng], f32)
            nc.gpsimd.dma_start(xT, x_d[:, :])
            sc_sb = singles.tile([P, n_chunks, oc_n, gpc, 2], f32)
            nc.gpsimd.dma_start(sc_sb, sc_d[:, :, :, :, :])

            # Dekker split: x2[:, :, 0] = bf16(x), x2[:, :, 1] = bf16(x - hi)
            x2 = singles.tile([P, ng, 2], bf16)
            xhi32 = singles.tile([P, ng], f32)
            xlo32 = singles.tile([P, ng], f32)
            nc.vector.tensor_copy(out=x2[:, :, 0], in_=xT)
            nc.vector.tensor_copy(out=xhi32, in_=x2[:, :, 0])
            nc.vector.tensor_tensor(xlo32, xT, xhi32, mybir.AluOpType.subtract)
            nc.vector.tensor_copy(out=x2[:, :, 1], in_=xlo32)

            out_acc = [singles.tile([P, oc_n], f32, name=f"oacc{i}") for i in (0, 1)]

            for ch in range(n_chunks):
                # ---- weight DMAs for this chunk
                wfa = wapool.tile([P, gp_a, per_out], f32, tag="wfa")
                nc.sync.dma_start(wfa, wt[:, ch, 0:gp_a, :])
                wfb = wbpool.tile([P, gp_b, per_out], f32, tag="wfb")
                for k in range(8):
                    nc.gpsimd.dma_start(
                        wfb[15 * k : 15 * k + 15, :, :],
                        wt[15 * k : 15 * k + 15, ch, gp_a:gpc, :],
                    )
                nc.gpsimd.dma_start(
                    wfb[120:128, :, :],
                    wt[120:128, ch, gp_a:gpc, :],
                    max_dma_last_dim=512,
                )

                # ---- quantize (fp32 -> round-half-even ints in bf16)
                qwa = qapool.tile([P, gp_a, per_out], bf16, tag="qwa")
                nc.vector.tensor_scalar(
                    out=qwa, in0=wfa,
                    scalar1=float(MAGIC), scalar2=-float(MAGIC),
                    op0=add, op1=add,
                )
                qwb = qbpool.tile([P, gp_b, per_out], bf16, tag="qwb")
                nc.vector.tensor_scalar(
                    out=qwb, in0=wfb,
                    scalar1=float(MAGIC), scalar2=-float(MAGIC),
                    op0=add, op1=add,
                )

                # ---- matmuls: acc[:, oc, gp, :] += qw[:, gp, oc-block].T @ x2
                acc = psum.tile([P, oc_n, gpc, 2], f32, tag="acc", name=f"acc{ch}")
                for gp in range(gpc):
                    qw, k = (qwa, gp) if gp < gp_a else (qwb, gp - gp_a)
                    g = ch * gpc + gp
                    for oc in range(oc_n):
                        nc.tensor.matmul(
                            acc[:, oc, gp, :],
                            lhsT=qw[:, k, oc * P : (oc + 1) * P],
                            rhs=x2[:, g, :],
                            start=True,
                            stop=True,
                        )

                # ---- chunk epilogue: out_acc += sum_{gp,j}(acc * scales)
                y = epool.tile([P, oc_n, gpc, 2], f32, tag="y")
                nc.vector.tensor_tensor(y, acc[:, :, :, :], sc_sb[:, ch], mult)
                dst = out_acc[(ch + 1) % 2]
                if ch == 0:
                    nc.vector.reduce_sum(out=dst, in_=y, axis=mybir.AxisListType.XY)
                else:
                    part = epool.tile([P, oc_n], f32, tag="part")
                    nc.vector.reduce_sum(out=part, in_=y, axis=mybir.AxisListType.XY)
                    nc.vector.tensor_tensor(dst, out_acc[ch % 2], part, add)

            nc.sync.dma_start(out_d[:, :], out_acc[n_chunks % 2])

    return _split_multi_waits(nc) if split_waits else nc


def _prep_in_maps(x, weights, scales):
    """Pure-layout host prep: shard + permute per core."""
    x = np.asarray(x, dtype=np.float32)
    weights = np.asarray(weights, dtype=np.float32)
    scales = np.asarray(scales, dtype=np.float32)

    gpc = 8
    n_chunks = NUM_GROUPS // gpc
    oc_n = PER_OUT // P

    xT = np.ascontiguousarray(x.reshape(NUM_GROUPS, P).T)  # [128, 64]
    in_maps = []
    for c in range(N_CORES):
        sl = slice(c * PER_OUT, (c + 1) * PER_OUT)
        w_sl = weights[sl]  # [1024, 8192]
        # wt[c, ch, gp, o] = W[o, ch*1024 + gp*128 + c]
        wt = np.ascontiguousarray(
            w_sl.reshape(PER_OUT, n_chunks, gpc, P).transpose(3, 1, 2, 0)
        )
        s_sl = scales[sl]  # [1024, 64]
        # sc[p, ch, oc, gp, j] = scales[oc*128 + p, ch*gpc + gp]
        sc = s_sl.reshape(oc_n, P, n_chunks, gpc).transpose(1, 2, 0, 3)
        sc2 = np.ascontiguousarray(
            np.broadcast_to(sc[..., None], (P, n_chunks, oc_n, gpc, 2))
        )
        in_maps.append({"wt": wt, "x": xT, "scales": sc2})
    return in_maps


def kernel(x, weights, scales):
    from concourse import bass_utils

    if "nc" not in _cache:
        _cache["nc"] = build_nc()
    nc = _cache["nc"]

    in_maps = _prep_in_maps(x, weights, scales)
    res = bass_utils.run_bass_kernel_spmd(nc, in_maps, core_ids=list(range(N_CORES)))
    # out[p, oc] -> result[oc*128 + p]
    return np.concatenate(
        [res.results[c]["out"].T.reshape(-1) for c in range(N_CORES)]
    ).astype(np.float32)


# revision 9
# speedup vs baseline: 1.0576x; 1.0576x over previous
"""Group-quantized linear (fake int4 per-group dequant) GEMV on 8 Trainium2 cores.

Reference computation (all fp32):
    qw = round_half_even(clip(W, -8, 7))            # W in [-8, 7) so clip is identity
    out = (qw.reshape(O, 64, 128) * scales[:, :, None]).reshape(O, O) @ x

Sharding: column-parallel — each core owns a 1024-row slice of W/scales,
x replicated, outputs concatenated (per the tensor-parallel hint).  All
host-side work is pure layout: the per-core weight slice ships fully
permuted to the SBUF tile order [c(128), ch(8), gp(8), o(1024)] so weight
DMA descriptors are large contiguous reads, x ships pre-transposed
[128, 64], scales ship pre-arranged (and duplicated over the hi/lo axis),
and the [128, 8] result is un-permuted on the host — no on-device
transposes at all.

DMA-engine load balancing (all on the ONE sync-engine ring — concurrent
rings make the engines interleave queues and collapse throughput):
descriptors round-robin over the 16 DMA engines E64..E79 and the pointer
RESETS to E64 at every dma_start (measured).  On this part engine E79 runs
~17% slower than the rest, so a uniform split leaves a 13 us straggler
tail.  Each chunk therefore ships as:
  A:  gp 0..5, all 128 partitions       -> 128 descs x 24 KiB, uniform
  B0..B7: gp 6..7, partitions 15k..15k+14 -> 15 descs x 8 KiB each: E79 gets
      NONE of these (desc index mod 16 never reaches 15)
  C:  gp 6..7, partitions 120..127, max_dma_last_dim=512 -> 32 descs x 2 KiB,
      uniform
Per-chunk bytes: E64..E78 ~260 KiB (10.2 us at their 26 GB/s), E79
~196 KiB (9.1 us at its 21.6 GB/s) — near-balanced, ~81 us stream.

Per-core pipeline (device):
  DVE   : quantize via the fp32 magic-number trick (w + 1.5*2^23) - 1.5*2^23
          == round-half-even exactly for |w| < 2^22, cast to bf16 (exact for
          ints in [-8, 7]); one tensor_scalar per A/B part
  PE    : per (group g, out-chunk oc) matmul acc[:, oc, gp, :2] =
          qwT[128c, 128o].T @ x2[128c, 2] where x2 = [x_hi | x_lo] bf16
          Dekker split of x (fp32-accurate), fp32 PSUM; acc is one ping-pong
          PSUM bank per chunk
  DVE   : per chunk, THREE ops covering all out-chunks at once:
          y[128, oc, gp, 2] = acc * sc2[:, ch]      (PSUM read, one TT)
          partial[128, oc]  = reduce_sum(y, XY)     (hi/lo + group reduction)
          out_acc           = out_acc + partial     (ping-pong buffers)

HBM traffic/core = 32 MiB weights; balanced engines => ~80 us stream floor.
"""

import numpy as np

IN_DIM = 8192
OUT_DIM = 8192
NUM_GROUPS = 64
GROUP_SIZE = 128  # IN_DIM // NUM_GROUPS
N_CORES = 8
PER_OUT = OUT_DIM // N_CORES  # 1024
P = 128

MAGIC = np.float32(12582912.0)  # 1.5 * 2**23: (w + MAGIC) - MAGIC == rint(w)

_cache = {}


def _split_multi_waits(nc):
    """walrus in this container accepts only ONE sync-wait per instruction;
    Tile's tail drain carries one per producer proc. Hoist extras onto
    same-engine NoOps placed immediately before — identical semantics for an
    in-order sequencer."""
    import concourse.mybir as mybir

    uid = 0
    for f in nc.m.functions:
        for blk in f.blocks:
            insts = blk.instructions
            if not any(
                i.sync_info is not None
                and i.sync_info.on_wait
                and len(i.sync_info.on_wait) > 1
                for i in insts
            ):
                continue
            new_insts = []
            for inst in insts:
                si = inst.sync_info
                if si is not None and si.on_wait and len(si.on_wait) > 1:
                    waits = list(si.on_wait)
                    for w in waits[:-1]:
                        uid += 1
                        new_insts.append(
                            mybir.InstNoOp(
                                name=f"I-waitsplit-{uid}",
                                engine=inst.engine,
                                ins=[],
                                outs=[],
                                sync_info=mybir.SyncInfo(on_wait=[w], on_update=[]),
                            )
                        )
                    inst.sync_info = mybir.SyncInfo(
                        on_wait=[waits[-1]], on_update=si.on_update
                    )
                new_insts.append(inst)
            blk.instructions = new_insts
    return nc


def build_nc(
    in_dim=IN_DIM,
    per_out=PER_OUT,
    num_groups=NUM_GROUPS,
    groups_per_chunk=8,
    gp_a=6,  # groups in the uniform A part; gpc - gp_a in the E79-skipping B part
    wa_bufs=4,
    wb_bufs=4,
    split_waits=True,
):
    import concourse.bass as bass
    import concourse.mybir as mybir
    import concourse.tile as tile

    f32 = mybir.dt.float32
    bf16 = mybir.dt.bfloat16
    add = mybir.AluOpType.add
    mult = mybir.AluOpType.mult

    ng = num_groups
    gpc = groups_per_chunk
    n_chunks = ng // gpc
    oc_n = per_out // P  # out-chunks of 128
    gp_b = gpc - gp_a
    assert ng % gpc == 0 and per_out % P == 0 and in_dim == ng * GROUP_SIZE

    nc = bass.Bass()
    # host-permuted weights: wt[c, ch, gp, o] = W[o, ch*1024 + gp*128 + c]
    wt = nc.dram_tensor("wt", [P, n_chunks, gpc, per_out], f32, kind="ExternalInput")
    # host-transposed x: x[c, g] = x_full[g*128 + c]
    x_d = nc.dram_tensor("x", [P, ng], f32, kind="ExternalInput")
    # host-arranged scales, duplicated over the hi/lo axis:
    # sc[p, ch, oc, gp, j] = scales[oc*128 + p, ch*gpc + gp]
    sc_d = nc.dram_tensor("scales", [P, n_chunks, oc_n, gpc, 2], f32, kind="ExternalInput")
    # out[p, oc] = result[oc*128 + p]; host un-permutes
    out_d = nc.dram_tensor("out", [P, oc_n], f32, kind="ExternalOutput")

    with tile.TileContext(nc) as tc:
        with (
            tc.tile_pool(name="singles", bufs=1) as singles,
            tc.tile_pool(name="wa", bufs=wa_bufs) as wapool,
            tc.tile_pool(name="wb", bufs=wb_bufs) as wbpool,
            tc.tile_pool(name="qa", bufs=2) as qapool,
            tc.tile_pool(name="qb", bufs=2) as qbpool,
            tc.tile_pool(name="ep", bufs=2) as epool,
            tc.tile_pool(name="psum", bufs=2, space="PSUM") as psum,
        ):
            xT = singles.tile([P, ng], f32)
            sc_sb = singles.tile([P, n_chunks, oc_n, gpc, 2], f32)
            x2 = singles.tile([P, ng, 2], bf16)
            xhi32 = singles.tile([P, ng], f32)
            xlo32 = singles.tile([P, ng], f32)
            out_acc = [singles.tile([P, oc_n], f32, name=f"oacc{i}") for i in (0, 1)]

            for ch in range(n_chunks):
                # ---- weight DMAs for this chunk (all on the sync ring)
                wfa = wapool.tile([P, gp_a, per_out], f32, tag="wfa")
                nc.sync.dma_start(wfa, wt[:, ch, 0:gp_a, :])
                if ch == 0:
                    # tiny x/scales loads slot in right after the first big
                    # weight DMA; Dekker-split x on the DVE
                    nc.sync.dma_start(xT, x_d[:, :])
                    nc.sync.dma_start(sc_sb, sc_d[:, :, :, :, :])
                    nc.vector.tensor_copy(out=x2[:, :, 0], in_=xT)
                    nc.vector.tensor_copy(out=xhi32, in_=x2[:, :, 0])
                    nc.vector.tensor_tensor(
                        xlo32, xT, xhi32, mybir.AluOpType.subtract
                    )
                    nc.vector.tensor_copy(out=x2[:, :, 1], in_=xlo32)
                wfb = wbpool.tile([P, gp_b, per_out], f32, tag="wfb")
                for k in range(8):
                    nc.sync.dma_start(
                        wfb[15 * k : 15 * k + 15, :, :],
                        wt[15 * k : 15 * k + 15, ch, gp_a:gpc, :],
                    )
                nc.sync.dma_start(
                    wfb[120:128, :, :],
                    wt[120:128, ch, gp_a:gpc, :],
                    max_dma_last_dim=512,
                )

                # ---- quantize (fp32 -> round-half-even ints in bf16)
                qwa = qapool.tile([P, gp_a, per_out], bf16, tag="qwa")
                nc.vector.tensor_scalar(
                    out=qwa, in0=wfa,
                    scalar1=float(MAGIC), scalar2=-float(MAGIC),
                    op0=add, op1=add,
                )
                qwb = qbpool.tile([P, gp_b, per_out], bf16, tag="qwb")
                nc.vector.tensor_scalar(
                    out=qwb, in0=wfb,
                    scalar1=float(MAGIC), scalar2=-float(MAGIC),
                    op0=add, op1=add,
                )

                # ---- matmuls: acc[:, oc, gp, :] += qw[:, gp, oc-block].T @ x2
                acc = psum.tile([P, oc_n, gpc, 2], f32, tag="acc", name=f"acc{ch}")
                for gp in range(gpc):
                    qw, k = (qwa, gp) if gp < gp_a else (qwb, gp - gp_a)
                    g = ch * gpc + gp
                    for oc in range(oc_n):
                        nc.tensor.matmul(
                            acc[:, oc, gp, :],
                            lhsT=qw[:, k, oc * P : (oc + 1) * P],
                            rhs=x2[:, g, :],
                            start=True,
                            stop=True,
                        )

                # ---- chunk epilogue: out_acc += sum_{gp,j}(acc * scales)
                y = epool.tile([P, oc_n, gpc, 2], f32, tag="y")
                nc.vector.tensor_tensor(y, acc[:, :, :, :], sc_sb[:, ch], mult)
                dst = out_acc[(ch + 1) % 2]
                if ch == 0:
                    nc.vector.reduce_sum(out=dst, in_=y, axis=mybir.AxisListType.XY)
                else:
                    part = epool.tile([P, oc_n], f32, tag="part")
                    nc.vector.reduce_sum(out=part, in_=y, axis=mybir.AxisListType.XY)
                    nc.vector.tensor_tensor(dst, out_acc[ch % 2], part, add)

            nc.sync.dma_start(out_d[:, :], out_acc[n_chunks % 2])

    return _split_multi_waits(nc) if split_waits else nc


def _prep_in_maps(x, weights, scales):
    """Pure-layout host prep: shard + permute per core."""
    x = np.asarray(x, dtype=np.float32)
    weights = np.asarray(weights, dtype=np.float32)
    scales = np.asarray(scales, dtype=np.float32)

    gpc = 8
    n_chunks = NUM_GROUPS // gpc
    oc_n = PER_OUT // P

    xT = np.ascontiguousarray(x.reshape(NUM_GROUPS, P).T)  # [128, 64]
    in_maps = []
    for c in range(N_CORES):
        sl = slice(c * PER_OUT, (c + 1) * PER_OUT)
        w_sl = weights[sl]  # [1024, 8192]
        # wt[c, ch, gp, o] = W[o, ch*1024 + gp*128 + c]
        wt = np.ascontiguousarray(
            w_sl.reshape(PER_OUT, n_chunks, gpc, P).transpose(3, 1, 2, 0)
        )
        s_sl = scales[sl]  # [1024, 64]
        # sc[p, ch, oc, gp, j] = scales[oc*128 + p, ch*gpc + gp]
        sc = s_sl.reshape(oc_n, P, n_chunks, gpc).transpose(1, 2, 0, 3)
        sc2 = np.ascontiguousarray(
            np.broadcast_to(sc[..., None], (P, n_chunks, oc_n, gpc, 2))
        )
        in_maps.append({"wt": wt, "x": xT, "scales": sc2})
    return in_maps


def kernel(x, weights, scales):
    from concourse import bass_utils

    if "nc" not in _cache:
        _cache["nc"] = build_nc()
    nc = _cache["nc"]

    in_maps = _prep_in_maps(x, weights, scales)
    res = bass_utils.run_bass_kernel_spmd(nc, in_maps, core_ids=list(range(N_CORES)))
    # out[p, oc] -> result[oc*128 + p]
    return np.concatenate(
        [res.results[c]["out"].T.reshape(-1) for c in range(N_CORES)]
    ).astype(np.float32)
